# revision 19
# baseline (speedup 1.0000x reference)
"""CrossLayer (BatchNorm1d + rank-1 cross) Trainium2 Bass kernel.

Math (B=16384, D=1024):
    mean = x.mean(0); var = (x*x).mean(0) - mean^2
    scale = gamma / sqrt(var + EPS)                    (colA)
    xbn   = x * colA + (beta - mean*colA)
    s     = xbn @ w = x*colA*w summed over d + c0,  c0 = sum((beta - mean*colA) * w)
    out   = x0 * s[:, None] + bias + xbn
          = x0 * s[:, None] + x*colA + colC,        colC = bias + beta - mean*colA

Sharding: data-parallel over batch (2048 rows/core); BatchNorm partial sums
(sum, sumsq per column) are AllReduce'd across the 8 cores.

Device layout is TRANSPOSED (d on partitions, b on free dim), produced on the
host: stats become free-dim reductions (DVE reduce / ACT Square+accum), the
xbn transform becomes a per-partition scale+bias on the scalar engine, and
s = sum_d cw[d]*xT[d,b] is a natural partition-dim contraction on the PE.
"""

import numpy as np

import concourse.bass as bass
import concourse.tile as tile
from concourse import bacc, mybir
from concourse.bass_utils import run_bass_kernel_spmd

N_CORES = 8
B, D = 16384, 1024
B_LOC = B // N_CORES  # 2048
DC = D // 128  # 8 d-chunks of 128 partitions
EPS = 1e-8
F32 = mybir.dt.float32
F32R = mybir.dt.float32r
AF = mybir.ActivationFunctionType
OP = mybir.AluOpType

_built = {}


def _build(no_cc=False):
    nc = bacc.Bacc(
        "TRN2", target_bir_lowering=False, debug=False, num_devices=N_CORES
    )

    xT = nc.dram_tensor("xT", [D, B_LOC], F32, kind="ExternalInput")
    x0T = nc.dram_tensor("x0T", [D, B_LOC], F32, kind="ExternalInput")
    g8 = nc.dram_tensor("g8", [128, DC], F32, kind="ExternalInput")
    be8 = nc.dram_tensor("be8", [128, DC], F32, kind="ExternalInput")
    w8 = nc.dram_tensor("w8", [128, DC], F32, kind="ExternalInput")
    bi8 = nc.dram_tensor("bi8", [128, DC], F32, kind="ExternalInput")
    outT = nc.dram_tensor("outT", [D, B_LOC], F32, kind="ExternalOutput")

    cc_in = nc.dram_tensor("cc_in", [128, 2 * DC], F32)
    cc_out = nc.dram_tensor(
        "cc_out", [128, 2 * DC], F32, addr_space="Local" if no_cc else "Shared"
    )

    with tile.TileContext(nc) as tc:
        with (
            tc.tile_pool(name="xt", bufs=DC) as xt_pool,
            tc.tile_pool(name="x0t", bufs=DC) as x0t_pool,
            tc.tile_pool(name="junk", bufs=2) as junk_pool,
            tc.tile_pool(name="small", bufs=1) as small,
            tc.tile_pool(name="psum", bufs=1, space="PSUM") as psum,
        ):
            # ---- persistent tiles ----
            xt = [xt_pool.tile([128, B_LOC], F32, tag="xt", name=f"xt{j}") for j in range(DC)]
            x0t = [x0t_pool.tile([128, B_LOC], F32, tag="x0t", name=f"x0t{j}") for j in range(DC)]
            stats = small.tile([128, 2 * DC], F32)  # cols 0..7 sum, 8..15 sumsq
            gstats = small.tile([128, 2 * DC], F32)
            g8s = small.tile([128, DC], F32)
            be8s = small.tile([128, DC], F32)
            w8s = small.tile([128, DC], F32)
            bi8s = small.tile([128, DC], F32)
            bb8 = small.tile([128, DC], F32)
            mean8 = small.tile([128, DC], F32)
            e8 = small.tile([128, DC], F32)
            msq8 = small.tile([128, DC], F32)
            var8 = small.tile([128, DC], F32)
            std8 = small.tile([128, DC], F32)
            rstd8 = small.tile([128, DC], F32)
            colA8 = small.tile([128, DC], F32)
            mc8 = small.tile([128, DC], F32)
            colC8 = small.tile([128, DC], F32)
            q8 = small.tile([128, DC], F32)
            ones = small.tile([128, 1], F32)
            epsv = small.tile([128, 1], F32)
            ones_row = small.tile([1, 128], F32)
            c0sb = small.tile([1, 1], F32)
            s_sb = small.tile([1, B_LOC], F32)

            ps_c0 = psum.tile([1, DC], F32)
            ps_sb = psum.tile([128, B_LOC], F32)  # broadcast s (4 banks)

            # ---- phase 0: input DMAs (x first, then x0; SP queue is FIFO) ----
            for j in range(DC):
                nc.sync.dma_start(xt[j][:], xT[bass.ts(j, 128), :])
            for j in range(DC):
                nc.sync.dma_start(x0t[j][:], x0T[bass.ts(j, 128), :])
            # params
            nc.gpsimd.dma_start(g8s[:], g8[:])
            nc.gpsimd.dma_start(be8s[:], be8[:])
            nc.gpsimd.dma_start(w8s[:], w8[:])
            nc.gpsimd.dma_start(bi8s[:], bi8[:])
            nc.gpsimd.memset(ones[:], 1.0)
            nc.gpsimd.memset(epsv[:], EPS)
            nc.gpsimd.memset(ones_row[:], 1.0)
            nc.vector.tensor_add(bb8[:], be8s[:], bi8s[:])

            # ---- phase 1: local stats ----
            for j in range(DC):
                nc.vector.tensor_reduce(
                    stats[:, j : j + 1], xt[j][:], axis=mybir.AxisListType.X, op=OP.add
                )
                jk = junk_pool.tile([128, B_LOC], F32, tag="junk", name=f"junk{j}")
                nc.scalar.activation(
                    jk[:], xt[j][:], AF.Square, accum_out=stats[:, DC + j : DC + j + 1]
                )

            # ---- allreduce of [sum, sumsq] ----
            nc.gpsimd.dma_start(cc_in[:], stats[:])
            if no_cc:
                # TimelineSim-compatible stand-in (single-core, no collectives):
                # timing-equivalent dram->dram copy, numerically WRONG (no 8x sum)
                nc.gpsimd.dma_start(cc_out[:], cc_in[:])
            else:
                nc.gpsimd.collective_compute(
                    "AllReduce",
                    OP.add,
                    replica_groups=[list(range(N_CORES))],
                    ins=[cc_in[:]],
                    outs=[cc_out[:]],
                )
            nc.gpsimd.dma_start(gstats[:], cc_out[:])

            # ---- chain: per-column params, all [128, 8] ----
            inv_b = 1.0 / float(B)
            nc.vector.tensor_scalar_mul(mean8[:], gstats[:, 0:DC], inv_b)
            nc.vector.tensor_scalar_mul(e8[:], gstats[:, DC : 2 * DC], inv_b)
            nc.vector.tensor_mul(msq8[:], mean8[:], mean8[:])
            nc.vector.tensor_sub(var8[:], e8[:], msq8[:])
            nc.scalar.activation(std8[:], var8[:], AF.Sqrt, bias=epsv[:])
            nc.vector.reciprocal(rstd8[:], std8[:])
            nc.vector.tensor_mul(colA8[:], rstd8[:], g8s[:])
            nc.vector.tensor_mul(mc8[:], mean8[:], colA8[:])
            nc.vector.tensor_sub(colC8[:], bb8[:], mc8[:])
            # the dot runs against xbn+bias, so correct s by c0 = -sum(bias*w)
            nc.vector.tensor_mul(q8[:], bi8s[:], w8s[:])
            nc.tensor.matmul(ps_c0[:], ones[:], q8[:], start=True, stop=True)
            nc.vector.tensor_reduce(
                c0sb[:], ps_c0[:], axis=mybir.AxisListType.X, op=OP.add, negate=True
            )

            # ---- combine, pipelined per 512-col chunk ----
            # xt[j] := t = x*colA + colC   (= xbn + bias; in place, ACT scale+bias)
            # s' = sum_d t[d,b]*w[d]  (PE dot)  →  s = s' + c0  (c0 = -sum(bias*w))
            # out = x0*s + t
            for c in range(B_LOC // 512):
                cs = bass.ts(c, 512)
                for j in range(DC):
                    nc.scalar.activation(
                        xt[j][:, cs],
                        xt[j][:, cs],
                        AF.Identity,
                        scale=colA8[:, j : j + 1],
                        bias=colC8[:, j : j + 1],
                    )
                ps_s = psum.tile([1, 512], F32, tag="dot", bufs=2, name=f"ps_s{c}")
                for j in range(DC):
                    nc.tensor.matmul(
                        ps_s[:],
                        w8s[:, j : j + 1],
                        xt[j][:, cs],
                        start=(j == 0),
                        stop=(j == DC - 1),
                    )
                # s (+c0) to SBUF row
                nc.scalar.activation(s_sb[:, cs], ps_s[:], AF.Identity, bias=c0sb[:])
                # broadcast s over partitions: ones_row[1,128].T @ s_sb[1,:]
                nc.tensor.matmul(
                    ps_sb[:, cs], ones_row[:], s_sb[:, cs], start=True, stop=True
                )
                for j in range(DC):
                    # m = x0 * s (in place over x0t; s broadcast read from PSUM)
                    nc.vector.tensor_mul(x0t[j][:, cs], x0t[j][:, cs], ps_sb[:, cs])
                    # out = m + t (in place over xt)
                    nc.vector.tensor_add(xt[j][:, cs], xt[j][:, cs], x0t[j][:, cs])
            for j in range(DC):
                nc.scalar.dma_start(outT[bass.ts(j, 128), :], xt[j][:])

    nc.compile()
    return nc


def _get_nc(no_cc=False):
    key = "nc_nocc" if no_cc else "nc"
    if key not in _built:
        _built[key] = _build(no_cc=no_cc)
    return _built[key]


def _p8(p):
    # param [D] -> [128, 8] with p8[r, j] = p[j*128 + r]
    return np.ascontiguousarray(np.asarray(p, dtype=np.float32).reshape(DC, 128).T)


def kernel(x, x0, gamma, beta, weight, bias):
    nc = _get_nc()
    x = np.asarray(x, dtype=np.float32)
    x0 = np.asarray(x0, dtype=np.float32)
    g8 = _p8(gamma)
    be8 = _p8(beta)
    w8 = _p8(weight)
    bi8 = _p8(bias)

    in_maps = []
    for c in range(N_CORES):
        sl = slice(c * B_LOC, (c + 1) * B_LOC)
        in_maps.append(
            {
                "xT": np.ascontiguousarray(x[sl].T),
                "x0T": np.ascontiguousarray(x0[sl].T),
                "g8": g8,
                "be8": be8,
                "w8": w8,
                "bi8": bi8,
            }
        )

    res = run_bass_kernel_spmd(nc, in_maps, core_ids=list(range(N_CORES)))
    out = np.empty((B, D), dtype=np.float32)
    for c in range(N_CORES):
        out[c * B_LOC : (c + 1) * B_LOC] = res.results[c]["outT"].T
    return out
